# revision 1
# baseline (speedup 1.0000x reference)
"""AdaptiveTripletLoss on 8 TRN2 NeuronCores.

Device: the compute-dominant Gram matrix G = E @ E^T (4096x4096x2048,
68.7 GFLOP) in bf16 on the PE, f32 PSUM. Symmetry-aware: only the 36
upper-triangular 512x512 blocks are computed, slot-packed 5 per core
(4 cores carry one duplicate pad block). Host mirrors the blocks,
then does masks/counts, order-statistic selection (value-stable under
bf16 distance jitter), exact d_ap/d_an norms and the final masked mean.
"""

import os

import numpy as np
import ml_dtypes

N, D = 4096, 2048
NUM_IDS = 512
N_CORES = 8
MARGIN = 0.3
RATIOS = (0.3, 0.4, 0.3)
EPS = 1e-6

B = 512           # block edge
NB = N // B       # 8x8 block grid
SLOTS = 5         # blocks per core (36 real + 4 pad)
KT = D // 128     # 16 k-tiles

LAST_EXEC_NS = None

_BF16 = ml_dtypes.bfloat16


def _block_assignment():
    """Each core gets 3 blocks sharing row-group A plus 2 sharing row-group
    B (36 real upper-tri blocks + 4 duplicate pads) so the per-core lhs
    input is just two 512-row slices instead of five."""
    c3 = [(0, [0, 1, 2]), (0, [3, 4, 5]), (1, [1, 2, 3]), (2, [2, 3, 4]),
          (2, [5, 6, 7]), (3, [3, 4, 5]), (4, [4, 5, 6]), (5, [5, 6, 7])]
    c2 = [(0, [6, 7]), (1, [4, 5]), (1, [6, 7]), (3, [6, 7]),
          (6, [6, 7]), (4, [7, 7]), (7, [7, 7]), (0, [0, 1])]
    per_core = []
    for (ra, cas), (rb, cbs) in zip(c3, c2):
        per_core.append([(ra, c) for c in cas] + [(rb, c) for c in cbs])
    return per_core


_ASSIGN = _block_assignment()


def _build_gram_kernel():
    import concourse.bacc as bacc
    import concourse.tile as tile
    from concourse import mybir

    nc = bacc.Bacc(None, target_bir_lowering=False)

    f32 = mybir.dt.float32
    bf16 = mybir.dt.bfloat16

    fp8 = mybir.dt.float8e4
    W = SLOTS * B  # 2560 packed rhs columns
    lhsP = nc.declare_dram_parameter("lhsP", [D, 2 * B], fp8, isOutput=False)
    rhsP = nc.declare_dram_parameter("rhsP", [D, W], fp8, isOutput=False)
    out = nc.declare_dram_parameter("out", [W, B], bf16, isOutput=True)

    GRP = (0, 0, 0, 1, 1)  # slot -> lhs row-group
    TT = KT // 2  # 8 DoubleRow steps, each contracting 256 k-rows

    with tile.TileContext(nc) as tc:
        with (
            tc.tile_pool(name="lhs_p", bufs=1) as lhs_pool,
            tc.tile_pool(name="rhs_p", bufs=1) as rhs_pool,
            tc.tile_pool(name="psum", bufs=8, space="PSUM") as psum_pool,
            tc.tile_pool(name="outp", bufs=8) as out_pool,
        ):
            lhs_t, rhs_t = {}, {}

            # Tiles are [128, 2, B] fp8: dim1 holds the adjacent k-tile
            # pair one DoubleRow matmul contracts in a single pass.
            def load_lhs(issuer, g, t):
                tl = lhs_pool.tile([128, 2, B], fp8, tag=f"l{g}_{t}")
                for i in range(2):
                    k = 2 * t + i
                    issuer.dma_start(
                        tl[:, i, :],
                        lhsP[k * 128:(k + 1) * 128, g * B:(g + 1) * B],
                    )
                lhs_t[(g, t)] = tl

            def load_rhs(issuer, s, t):
                tl = rhs_pool.tile([128, 2, B], fp8, tag=f"r{s}_{t}")
                for i in range(2):
                    k = 2 * t + i
                    issuer.dma_start(
                        tl[:, i, :],
                        rhsP[k * 128:(k + 1) * 128, s * B:(s + 1) * B],
                    )
                rhs_t[(s, t)] = tl

            # Critical prefix: slot 0's lhs/rhs pairs go on the two fast
            # HWDGE queues only (~600 ns/issue) so its chain starts
            # streaming ~10 us in; gpsimd's slower SWDGE (~1 us/issue)
            # carries only later-need chunks. Remaining chunks round-robin
            # across all three queues in slot-major (need-by) order.
            for t in range(TT):
                load_lhs(nc.sync, 0, t)
                load_rhs(nc.scalar, 0, t)

            rest = [nc.sync, nc.scalar, nc.gpsimd]
            ri = 0

            def nxt():
                nonlocal ri
                e = rest[ri % 3]
                ri += 1
                return e

            for s in (1, 2):
                for t in range(TT):
                    load_rhs(nxt(), s, t)
            for t in range(TT):
                load_lhs(nxt(), 1, t)
            for s in (3, 4):
                for t in range(TT):
                    load_rhs(nxt(), s, t)

            for s in range(SLOTS):
                for m in range(B // 128):
                    ps = psum_pool.tile([128, B], f32)
                    for t in range(TT):
                        nc.tensor.matmul(
                            ps[:],
                            lhs_t[(GRP[s], t)][:, :, m * 128:(m + 1) * 128],
                            rhs_t[(s, t)][:],
                            start=(t == 0),
                            stop=(t == TT - 1),
                            perf_mode=mybir.MatmulPerfMode.DoubleRow,
                        )
                    ot = out_pool.tile([128, B], bf16)
                    nc.vector.tensor_copy(ot[:], ps[:])
                    r0 = s * B + m * 128
                    # Last slot's outputs drain in parallel across the
                    # by-then-idle queues instead of serializing ~3 us of
                    # transfers into the kernel tail on scalar alone.
                    if s == SLOTS - 1:
                        oeng = [nc.sync, nc.gpsimd, nc.sync, nc.gpsimd][m]
                    else:
                        oeng = nc.scalar
                    oeng.dma_start(out[r0:r0 + 128, :], ot[:])

    nc.compile()
    return nc


_NC_CACHE = None


def _run_gram(emb: np.ndarray) -> np.ndarray:
    """Run the 8-core symmetric Gram kernel; returns G = emb @ emb.T f32."""
    global _NC_CACHE, LAST_EXEC_NS
    from concourse.bass_utils import run_bass_kernel_spmd

    if _NC_CACHE is None:
        _NC_CACHE = _build_gram_kernel()
    nc = _NC_CACHE

    eT_bf = np.ascontiguousarray(emb.T).astype(ml_dtypes.float8_e4m3)
    in_maps = []
    for core in range(N_CORES):
        slots = _ASSIGN[core]
        ra, rb = slots[0][0], slots[3][0]
        lhs = np.concatenate(
            [eT_bf[:, ra * B:(ra + 1) * B], eT_bf[:, rb * B:(rb + 1) * B]],
            axis=1,
        )
        rhs = np.concatenate(
            [eT_bf[:, c * B:(c + 1) * B] for (r, c) in slots], axis=1
        )
        in_maps.append(
            {"lhsP": np.ascontiguousarray(lhs), "rhsP": np.ascontiguousarray(rhs)}
        )

    trace = bool(int(os.environ.get("KERNEL_TRACE", "0")))
    res = run_bass_kernel_spmd(
        nc, in_maps, core_ids=list(range(N_CORES)), trace=trace
    )
    if res.exec_time_ns is not None:
        LAST_EXEC_NS = res.exec_time_ns

    G = np.empty((N, N), dtype=np.float32)
    for core in range(N_CORES):
        o = np.asarray(res.results[core]["out"], dtype=np.float32)
        for s, (r, c) in enumerate(_ASSIGN[core]):
            blk = o[s * B:(s + 1) * B, :]
            G[r * B:(r + 1) * B, c * B:(c + 1) * B] = blk
            if r != c:
                G[c * B:(c + 1) * B, r * B:(r + 1) * B] = blk.T
    return G


def _sample_js(counts: np.ndarray, us: list) -> np.ndarray:
    """Replicate the reference's f32 sampling math. counts [N] int, us 3x[N]
    f32 uniforms. Returns j ranks [N, 3] int64 (rank into the masked sort)."""
    out = []
    for t, r in enumerate(RATIOS):
        cnt = np.maximum(
            np.int32(1),
            np.floor(counts.astype(np.float32) * np.float32(r)).astype(np.int32),
        )
        j = np.minimum((us[t] * cnt.astype(np.float32)).astype(np.int32), cnt - 1)
        out.append(j.astype(np.int64))
    return np.stack(out, axis=1)


def kernel(embeddings: np.ndarray, labels: np.ndarray) -> np.ndarray:
    emb = np.ascontiguousarray(np.asarray(embeddings, dtype=np.float32))
    lab = np.asarray(labels).astype(np.int64)

    G = _run_gram(emb)

    # Selection keys: within row i, ordering by (sq_j - 2 G[i,j]) equals
    # ordering by distance.
    sq = np.einsum("ij,ij->i", emb, emb).astype(np.float32)

    # Uniforms must match jax.random with key 42 bit-exactly.
    import jax

    with jax.default_device(jax.devices("cpu")[0]):
        skey = jax.random.key(42)
        keys = jax.random.split(skey, 6)
        us = [np.asarray(jax.random.uniform(k, (N,))) for k in keys]

    class_size = np.bincount(lab, minlength=NUM_IDS)
    pos_count = class_size[lab] - 1
    neg_count = N - class_size[lab]
    valid = (pos_count > 0) & (neg_count > 0)

    pos_js = _sample_js(pos_count, us[0:3])  # [N, 3]
    neg_js = _sample_js(neg_count, us[3:6])  # [N, 3]

    # Per-class member lists
    order = np.argsort(lab, kind="stable")
    sorted_lab = lab[order]
    starts = np.searchsorted(sorted_lab, np.arange(NUM_IDS), side="left")
    ends = np.searchsorted(sorted_lab, np.arange(NUM_IDS), side="right")

    pos_idx = np.zeros((N, 3), dtype=np.int64)
    neg_idx = np.zeros((N, 3), dtype=np.int64)
    INF = np.float32(np.inf)

    for i in range(N):
        li = lab[i]
        members = order[starts[li]:ends[li]]
        key_row = sq - 2.0 * G[i]  # f32 [N]
        if valid[i]:
            pos_members = members[members != i]
            pk = key_row[pos_members]
            po = np.argsort(pk, kind="stable")
            pos_idx[i] = pos_members[po[pos_js[i]]]
        # negatives: mask out own class and self
        nk = key_row.copy()
        nk[members] = INF
        nk[i] = INF
        kth = np.unique(neg_js[i])
        part = np.argpartition(nk, kth)
        neg_idx[i] = part[neg_js[i]]

    a = emb[:, None, :]
    p = emb[pos_idx]
    ng = emb[neg_idx]
    d_ap = np.sqrt(np.sum((a - p + np.float32(EPS)) ** 2, axis=-1))
    d_an = np.sqrt(np.sum((a - ng + np.float32(EPS)) ** 2, axis=-1))
    tri = np.maximum(d_ap - d_an + np.float32(MARGIN), np.float32(0.0))
    w = valid[:, None].astype(np.float32)
    denom = max(3.0 * float(valid.sum()), 1.0)
    loss = np.float32(np.sum(tri * w) / denom)
    return np.array(loss, dtype=np.float32)



# revision 8
# speedup vs baseline: 1.1922x; 1.1922x over previous
"""AdaptiveTripletLoss on 8 TRN2 NeuronCores.

Device: the compute-dominant Gram matrix G = E @ E^T (4096x4096x2048)
in fp8 DoubleRow on the PE, f32 PSUM. Cyclic symmetric assignment:
core w computes blocks (w, w+j mod 8) for j=0..4 with a SINGLE weight
row-block w, so the m-pass loop (m outer, t inner, 5 slots innermost)
shares one LDWEIGHTS across the 5 slots; a post-TileContext pass
dedupes the redundant Ldweights instructions (160 -> 32 weight loads).
Inputs arrive as 4 large pre-interleaved DMAs (1.25 MB each); outputs
drain per m-pass. Host mirrors the blocks, then does masks/counts,
order-statistic selection, exact d_ap/d_an norms and the masked mean.
"""

import os

import numpy as np
import ml_dtypes

N, D = 4096, 2048
NUM_IDS = 512
N_CORES = 8
MARGIN = 0.3
RATIOS = (0.3, 0.4, 0.3)
EPS = 1e-6

B = 512           # block edge
NB = N // B       # 8x8 block grid
SLOTS = 5         # blocks per core (36 real + 4 antipodal dups)
KT = D // 128     # 16 k-tiles
TT = KT // 2      # 8 DoubleRow steps, each contracting 256 k-rows
KC = 4            # input DMA chunks (each = 2 DoubleRow steps)

LAST_EXEC_NS = None


def _dedupe_ldweights(nc):
    """Remove Ldweights instructions identical to the immediately
    preceding one (same weights AP + mode): the PE array keeps the
    stationary operand loaded across matmuls, so consecutive matmuls
    sharing lhsT only need the first load."""
    removed = 0
    for fn in nc.m.functions:
        for blk in fn.blocks:
            il = blk.instructions
            prev_key = None
            prev_sync = None
            dels = []
            for idx in range(len(il)):
                ins = il[idx]
                op = ins.opcode
                if op == "Ldweights":
                    ap = ins.ins[0]
                    key = (
                        getattr(ap, "memref", None),
                        ap.offset,
                        str(ap.ap),
                        str(ins.perf_mode),
                        str(getattr(ins, "is_transpose", None)),
                        str(getattr(ins, "tile_position", None)),
                    )
                    sync = tuple(sorted(ins.sync_dependency_names()))
                    nosync = tuple(sorted(ins.nosync_dependency_names()))
                    if (
                        key == prev_key
                        and sync == prev_sync
                        and not nosync
                    ):
                        dels.append(idx)
                    else:
                        prev_key = key
                        prev_sync = sync
            for idx in reversed(dels):
                del il[idx]
            removed += len(dels)
    return removed


def _build_gram_kernel():
    import concourse.bacc as bacc
    import concourse.tile as tile
    from concourse import mybir

    nc = bacc.Bacc(None, target_bir_lowering=False)

    f32 = mybir.dt.float32
    bf16 = mybir.dt.bfloat16
    fp8 = mybir.dt.float8e4

    # Per-partition layout of the packed input: [KC, SLOTS, 2, 2, 512]
    # fp8 = 10240 B.  Chunk k holds DoubleRow steps t=2k, 2k+1 of all
    # 5 blocks; block 0 is also the core's weight row.
    blks = nc.declare_dram_parameter("blks", [KC * 128, SLOTS * 2048], fp8,
                                     isOutput=False)
    out = nc.declare_dram_parameter("out", [SLOTS * B, B], bf16, isOutput=True)

    with tile.TileContext(nc) as tc:
        with (
            tc.tile_pool(name="data", bufs=1) as dp,
            tc.tile_pool(name="psum", bufs=8, space="PSUM") as pp,
            tc.tile_pool(name="outp", bufs=8) as op,
        ):
            data = dp.tile([128, KC, SLOTS, 2, 2, 512], fp8, name="data")
            for k in range(KC):
                nc.sync.dma_start(
                    data[:, k, :, :, :, :],
                    blks[k * 128:(k + 1) * 128, :],
                )

            cast_eng = [nc.vector, nc.scalar, nc.vector, nc.scalar, nc.vector]
            dma_eng = [nc.scalar, nc.sync, nc.sync, nc.scalar, nc.sync]

            for m in range(4):
                ps = [pp.tile([128, B], f32, name="ps")
                      for s in range(SLOTS)]
                for t in range(TT):
                    k, u = t // 2, t % 2
                    w_ap = data[:, k, 0, u, :, m * 128:(m + 1) * 128]
                    for s in range(SLOTS):
                        nc.tensor.matmul(
                            ps[s][:],
                            w_ap,
                            data[:, k, s, u, :, :],
                            start=(t == 0),
                            stop=(t == TT - 1),
                            perf_mode=mybir.MatmulPerfMode.DoubleRow,
                        )
                for s in range(SLOTS):
                    ot = op.tile([128, B], bf16, name="ot")
                    ce = cast_eng[s]
                    if ce is nc.scalar:
                        ce.copy(ot[:], ps[s][:])
                    else:
                        ce.tensor_copy(ot[:], ps[s][:])
                    r0 = s * B + m * 128
                    dma_eng[s].dma_start(out[r0:r0 + 128, :], ot[:])

    _dedupe_ldweights(nc)
    nc.compile()
    return nc


_NC_CACHE = None


def _pack_core(eT8: np.ndarray, w: int) -> np.ndarray:
    """Pack core w's 5 cyclic blocks (w..w+4 mod 8) into the device
    layout [KC*128, SLOTS*2048]: row k*128+p holds [j, u, i, c] with
    k-row index (4k + 2u + i)*128 + p of eT."""
    # eT8 [D, N] -> [KC, u, i, p, block, c]
    R = eT8.reshape(KC, 2, 2, 128, NB, B)
    cols = [(w + j) % NB for j in range(SLOTS)]
    A = R[:, :, :, :, cols, :]          # [KC, 2, 2, 128, SLOTS, B]
    A = A.transpose(0, 3, 4, 1, 2, 5)   # [KC, 128, SLOTS, 2, 2, B]
    return np.ascontiguousarray(A).reshape(KC * 128, SLOTS * 2048)


def _run_gram(emb: np.ndarray) -> np.ndarray:
    """Run the 8-core symmetric Gram kernel; returns G = emb @ emb.T f32."""
    global _NC_CACHE, LAST_EXEC_NS
    from concourse.bass_utils import run_bass_kernel_spmd

    if _NC_CACHE is None:
        _NC_CACHE = _build_gram_kernel()
    nc = _NC_CACHE

    eT8 = np.ascontiguousarray(emb.T).astype(ml_dtypes.float8_e4m3)
    in_maps = [{"blks": _pack_core(eT8, w)} for w in range(N_CORES)]

    trace = bool(int(os.environ.get("KERNEL_TRACE", "0")))
    res = run_bass_kernel_spmd(
        nc, in_maps, core_ids=list(range(N_CORES)), trace=trace
    )
    if res.exec_time_ns is not None:
        LAST_EXEC_NS = res.exec_time_ns

    G = np.empty((N, N), dtype=np.float32)
    for w in range(N_CORES):
        o = np.asarray(res.results[w]["out"], dtype=np.float32)
        for j in range(SLOTS):
            if j == SLOTS - 1 and w >= NB // 2:
                continue  # antipodal dup; core w - 4 already wrote it
            c = (w + j) % NB
            blk = o[j * B:(j + 1) * B, :]
            G[w * B:(w + 1) * B, c * B:(c + 1) * B] = blk
            if c != w:
                G[c * B:(c + 1) * B, w * B:(w + 1) * B] = blk.T
    return G


def _sample_js(counts: np.ndarray, us: list) -> np.ndarray:
    """Replicate the reference's f32 sampling math. counts [N] int, us 3x[N]
    f32 uniforms. Returns j ranks [N, 3] int64 (rank into the masked sort)."""
    out = []
    for t, r in enumerate(RATIOS):
        cnt = np.maximum(
            np.int32(1),
            np.floor(counts.astype(np.float32) * np.float32(r)).astype(np.int32),
        )
        j = np.minimum((us[t] * cnt.astype(np.float32)).astype(np.int32), cnt - 1)
        out.append(j.astype(np.int64))
    return np.stack(out, axis=1)


def kernel(embeddings: np.ndarray, labels: np.ndarray) -> np.ndarray:
    emb = np.ascontiguousarray(np.asarray(embeddings, dtype=np.float32))
    lab = np.asarray(labels).astype(np.int64)

    G = _run_gram(emb)

    # Selection keys: within row i, ordering by (sq_j - 2 G[i,j]) equals
    # ordering by distance.
    sq = np.einsum("ij,ij->i", emb, emb).astype(np.float32)

    # Uniforms must match jax.random with key 42 bit-exactly.
    import jax

    with jax.default_device(jax.devices("cpu")[0]):
        skey = jax.random.key(42)
        keys = jax.random.split(skey, 6)
        us = [np.asarray(jax.random.uniform(k, (N,))) for k in keys]

    class_size = np.bincount(lab, minlength=NUM_IDS)
    pos_count = class_size[lab] - 1
    neg_count = N - class_size[lab]
    valid = (pos_count > 0) & (neg_count > 0)

    pos_js = _sample_js(pos_count, us[0:3])  # [N, 3]
    neg_js = _sample_js(neg_count, us[3:6])  # [N, 3]

    # Per-class member lists
    order = np.argsort(lab, kind="stable")
    sorted_lab = lab[order]
    starts = np.searchsorted(sorted_lab, np.arange(NUM_IDS), side="left")
    ends = np.searchsorted(sorted_lab, np.arange(NUM_IDS), side="right")

    pos_idx = np.zeros((N, 3), dtype=np.int64)
    neg_idx = np.zeros((N, 3), dtype=np.int64)
    INF = np.float32(np.inf)

    for i in range(N):
        li = lab[i]
        members = order[starts[li]:ends[li]]
        key_row = sq - 2.0 * G[i]  # f32 [N]
        if valid[i]:
            pos_members = members[members != i]
            pk = key_row[pos_members]
            po = np.argsort(pk, kind="stable")
            pos_idx[i] = pos_members[po[pos_js[i]]]
        # negatives: mask out own class and self
        nk = key_row.copy()
        nk[members] = INF
        nk[i] = INF
        kth = np.unique(neg_js[i])
        part = np.argpartition(nk, kth)
        neg_idx[i] = part[neg_js[i]]

    a = emb[:, None, :]
    p = emb[pos_idx]
    ng = emb[neg_idx]
    d_ap = np.sqrt(np.sum((a - p + np.float32(EPS)) ** 2, axis=-1))
    d_an = np.sqrt(np.sum((a - ng + np.float32(EPS)) ** 2, axis=-1))
    tri = np.maximum(d_ap - d_an + np.float32(MARGIN), np.float32(0.0))
    w = valid[:, None].astype(np.float32)
    denom = max(3.0 * float(valid.sum()), 1.0)
    loss = np.float32(np.sum(tri * w) / denom)
    return np.array(loss, dtype=np.float32)


# revision 13
# speedup vs baseline: 1.2262x; 1.0285x over previous
"""AdaptiveTripletLoss on 8 TRN2 NeuronCores.

Device: the compute-dominant Gram matrix G = E @ E^T (4096x4096x2048)
in fp8 DoubleRow on the PE, f32 PSUM. Cyclic symmetric assignment:
core w computes blocks (w, w+j mod 8) for j=0..4 with a SINGLE weight
row-block w; matmuls sharing one weight slice run back-to-back and a
post-TileContext pass dedupes the redundant Ldweights instructions.
The 20 (slot, m) accumulation units are scheduled in three PSUM-bank
cohorts (8/8/4); cohort 1 consumes the four streamed input chunks as
they arrive so the PE starts ~9us in and never starves. A few dummy
matmuls pre-warm the PE clock (HAM) during the first chunk's DMA.
Outputs drain per cohort as 512 KB batched DMAs with 4 KB contiguous
dram runs. Host mirrors the blocks, then does masks/counts,
order-statistic selection, exact d_ap/d_an norms and the masked mean.
"""

import os

import numpy as np
import ml_dtypes

N, D = 4096, 2048
NUM_IDS = 512
N_CORES = 8
MARGIN = 0.3
RATIOS = (0.3, 0.4, 0.3)
EPS = 1e-6

B = 512           # block edge
NB = N // B       # 8x8 block grid
SLOTS = 5         # blocks per core (36 real + 4 antipodal dups)
KT = D // 128     # 16 k-tiles
TT = KT // 2      # 8 DoubleRow steps, each contracting 256 k-rows
KC = 4            # input DMA chunks (each = 2 DoubleRow steps)
N_DUMMY = 6       # PE pre-warm matmuls during the first chunk's DMA

# (slot, m) accumulation units in drain order, split into PSUM cohorts.
U1 = [(0, 0), (1, 0), (2, 0), (3, 0), (4, 0), (0, 1), (1, 1), (2, 1)]
U2 = [(3, 1), (4, 1), (0, 2), (1, 2), (2, 2), (3, 2), (4, 2), (0, 3)]
U3 = [(1, 3), (2, 3), (3, 3), (4, 3)]
UNITS = U1 + U2 + U3

LAST_EXEC_NS = None


def _dedupe_ldweights(nc):
    """Remove Ldweights instructions identical to the immediately
    preceding one (same weights AP + mode): the PE array keeps the
    stationary operand loaded across matmuls, so consecutive matmuls
    sharing lhsT only need the first load."""
    removed = 0
    for fn in nc.m.functions:
        for blk in fn.blocks:
            il = blk.instructions
            prev_key = None
            prev_sync = None
            dels = []
            for idx in range(len(il)):
                ins = il[idx]
                if ins.opcode == "Ldweights":
                    ap = ins.ins[0]
                    key = (
                        getattr(ap, "memref", None),
                        ap.offset,
                        str(ap.ap),
                        str(ins.perf_mode),
                        str(getattr(ins, "is_transpose", None)),
                        str(getattr(ins, "tile_position", None)),
                    )
                    sync = tuple(sorted(ins.sync_dependency_names()))
                    nosync = tuple(sorted(ins.nosync_dependency_names()))
                    if key == prev_key and sync == prev_sync and not nosync:
                        dels.append(idx)
                    else:
                        prev_key = key
                        prev_sync = sync
            for idx in reversed(dels):
                del il[idx]
            removed += len(dels)
    return removed


def _build_gram_kernel():
    import concourse.bacc as bacc
    import concourse.tile as tile
    from concourse import mybir

    nc = bacc.Bacc(None, target_bir_lowering=False)

    f32 = mybir.dt.float32
    bf16 = mybir.dt.bfloat16
    fp8 = mybir.dt.float8e4
    DR = mybir.MatmulPerfMode.DoubleRow

    # Per-partition input layout: [KC, SLOTS, 2, 2, 512] fp8 = 10240 B.
    # Chunk k holds DoubleRow steps t=2k,2k+1 of all 5 blocks; block 0
    # is also the core's weight row.
    blks = nc.declare_dram_parameter("blks", [KC * 128, SLOTS * 2048], fp8,
                                     isOutput=False)
    # Output: row p holds all 20 units' 512 cols (unit-major), so a
    # 4-unit drain DMA writes 4 KB contiguous runs per dram row.
    out = nc.declare_dram_parameter("out", [128, len(UNITS) * B], bf16,
                                    isOutput=True)

    with tile.TileContext(nc) as tc:
        with (
            tc.tile_pool(name="data", bufs=1) as dp,
            tc.tile_pool(name="warm", bufs=1) as wp,
            tc.tile_pool(name="psum", bufs=8, space="PSUM") as pp,
            tc.tile_pool(name="outp", bufs=3) as op,
        ):
            data = dp.tile([128, KC, SLOTS, 2, 2, 512], fp8, name="data")
            for k in range(KC):
                nc.sync.dma_start(
                    data[:, k, :, :, :, :],
                    blks[k * 128:(k + 1) * 128, :],
                )

            # PE pre-warm: dummy matmuls on a zeroed tile while chunk 0
            # streams in; keeps HAM from throttling the first real mms.
            wl = wp.tile([128, 2, 128], fp8, name="wl")
            wr = wp.tile([128, 2, 512], fp8, name="wr")
            nc.gpsimd.memset(wl[:], 0.0)
            nc.gpsimd.memset(wr[:], 0.0)
            wps = pp.tile([128, B], f32, name="ps")
            for _ in range(N_DUMMY):
                nc.tensor.matmul(wps[:], wl[:], wr[:], start=True, stop=True,
                                 perf_mode=DR)

            ps = {}

            def mm(s, m, t, start, stop):
                k, u = t // 2, t % 2
                nc.tensor.matmul(
                    ps[(s, m)][:],
                    data[:, k, 0, u, :, m * 128:(m + 1) * 128],
                    data[:, k, s, u, :, :],
                    start=start, stop=stop, perf_mode=DR,
                )

            cast_i = 0

            def drain(units):
                """Cast `units` psums to bf16 and DMA them out in one
                batched transfer (4 KB contiguous per dram row)."""
                nonlocal cast_i
                g = op.tile([128, len(units), B], bf16, name="ot")
                for i, u in enumerate(units):
                    eng = (nc.vector, nc.scalar)[cast_i % 2]
                    if eng is nc.scalar:
                        eng.copy(g[:, i, :], ps[u][:])
                    else:
                        eng.tensor_copy(g[:, i, :], ps[u][:])
                    cast_i += 1
                u0 = UNITS.index(units[0])
                nc.scalar.dma_start(out[:, u0 * B:(u0 + len(units)) * B], g[:])

            # Cohort 1: consume chunks in arrival order.
            for u in U1:
                ps[u] = pp.tile([128, B], f32, name="ps")
            for c in range(KC):
                for tt in (2 * c, 2 * c + 1):
                    for m in (0, 1):
                        for (s, um) in U1:
                            if um == m:
                                mm(s, m, tt, start=(tt == 0),
                                   stop=(tt == TT - 1))
            drain(U1[0:4])
            drain(U1[4:8])

            # Cohort 2 (all chunks resident).
            for u in U2:
                ps[u] = pp.tile([128, B], f32, name="ps")
            for t in range(TT):
                for m in (1, 2, 3):
                    for (s, um) in U2:
                        if um == m:
                            mm(s, m, t, start=(t == 0), stop=(t == TT - 1))
            drain(U2[0:4])
            drain(U2[4:8])

            # Cohort 3.
            for u in U3:
                ps[u] = pp.tile([128, B], f32, name="ps")
            for t in range(TT):
                for (s, um) in U3:
                    mm(s, um, t, start=(t == 0), stop=(t == TT - 1))
            drain(U3[0:2])
            drain(U3[2:4])

    _dedupe_ldweights(nc)
    nc.compile()
    return nc


_NC_CACHE = None


def _pack_core(eT8: np.ndarray, w: int) -> np.ndarray:
    """Pack core w's 5 cyclic blocks (w..w+4 mod 8) into the device
    layout [KC*128, SLOTS*2048]: row k*128+p holds [j, u, i, c] with
    k-row index (4k + 2u + i)*128 + p of eT."""
    R = eT8.reshape(KC, 2, 2, 128, NB, B)
    cols = [(w + j) % NB for j in range(SLOTS)]
    A = R[:, :, :, :, cols, :]          # [KC, 2, 2, 128, SLOTS, B]
    A = A.transpose(0, 3, 4, 1, 2, 5)   # [KC, 128, SLOTS, 2, 2, B]
    return np.ascontiguousarray(A).reshape(KC * 128, SLOTS * 2048)


def _run_gram(emb: np.ndarray) -> np.ndarray:
    """Run the 8-core symmetric Gram kernel; returns G = emb @ emb.T f32."""
    global _NC_CACHE, LAST_EXEC_NS
    from concourse.bass_utils import run_bass_kernel_spmd

    if _NC_CACHE is None:
        _NC_CACHE = _build_gram_kernel()
    nc = _NC_CACHE

    eT8 = np.ascontiguousarray(emb.T).astype(ml_dtypes.float8_e4m3)
    in_maps = [{"blks": _pack_core(eT8, w)} for w in range(N_CORES)]

    trace = bool(int(os.environ.get("KERNEL_TRACE", "0")))
    res = run_bass_kernel_spmd(
        nc, in_maps, core_ids=list(range(N_CORES)), trace=trace
    )
    if res.exec_time_ns is not None:
        LAST_EXEC_NS = res.exec_time_ns

    G = np.empty((N, N), dtype=np.float32)
    for w in range(N_CORES):
        o = np.asarray(res.results[w]["out"], dtype=np.float32)
        o = o.reshape(128, len(UNITS), B)
        for ui, (s, m) in enumerate(UNITS):
            if s == SLOTS - 1 and w >= NB // 2:
                continue  # antipodal dup; core w - 4 already wrote it
            c = (w + s) % NB
            rows = slice(w * B + m * 128, w * B + (m + 1) * 128)
            blk = o[:, ui, :]
            G[rows, c * B:(c + 1) * B] = blk
            if c != w:
                G[c * B:(c + 1) * B, rows] = blk.T
    return G


def _sample_js(counts: np.ndarray, us: list) -> np.ndarray:
    """Replicate the reference's f32 sampling math. counts [N] int, us 3x[N]
    f32 uniforms. Returns j ranks [N, 3] int64 (rank into the masked sort)."""
    out = []
    for t, r in enumerate(RATIOS):
        cnt = np.maximum(
            np.int32(1),
            np.floor(counts.astype(np.float32) * np.float32(r)).astype(np.int32),
        )
        j = np.minimum((us[t] * cnt.astype(np.float32)).astype(np.int32), cnt - 1)
        out.append(j.astype(np.int64))
    return np.stack(out, axis=1)


def kernel(embeddings: np.ndarray, labels: np.ndarray) -> np.ndarray:
    emb = np.ascontiguousarray(np.asarray(embeddings, dtype=np.float32))
    lab = np.asarray(labels).astype(np.int64)

    G = _run_gram(emb)

    # Selection keys: within row i, ordering by (sq_j - 2 G[i,j]) equals
    # ordering by distance.
    sq = np.einsum("ij,ij->i", emb, emb).astype(np.float32)

    # Uniforms must match jax.random with key 42 bit-exactly.
    import jax

    with jax.default_device(jax.devices("cpu")[0]):
        skey = jax.random.key(42)
        keys = jax.random.split(skey, 6)
        us = [np.asarray(jax.random.uniform(k, (N,))) for k in keys]

    class_size = np.bincount(lab, minlength=NUM_IDS)
    pos_count = class_size[lab] - 1
    neg_count = N - class_size[lab]
    valid = (pos_count > 0) & (neg_count > 0)

    pos_js = _sample_js(pos_count, us[0:3])  # [N, 3]
    neg_js = _sample_js(neg_count, us[3:6])  # [N, 3]

    # Per-class member lists
    order = np.argsort(lab, kind="stable")
    sorted_lab = lab[order]
    starts = np.searchsorted(sorted_lab, np.arange(NUM_IDS), side="left")
    ends = np.searchsorted(sorted_lab, np.arange(NUM_IDS), side="right")

    pos_idx = np.zeros((N, 3), dtype=np.int64)
    neg_idx = np.zeros((N, 3), dtype=np.int64)
    INF = np.float32(np.inf)

    for i in range(N):
        li = lab[i]
        members = order[starts[li]:ends[li]]
        key_row = sq - 2.0 * G[i]  # f32 [N]
        if valid[i]:
            pos_members = members[members != i]
            pk = key_row[pos_members]
            po = np.argsort(pk, kind="stable")
            pos_idx[i] = pos_members[po[pos_js[i]]]
        # negatives: mask out own class and self
        nk = key_row.copy()
        nk[members] = INF
        nk[i] = INF
        kth = np.unique(neg_js[i])
        part = np.argpartition(nk, kth)
        neg_idx[i] = part[neg_js[i]]

    a = emb[:, None, :]
    p = emb[pos_idx]
    ng = emb[neg_idx]
    d_ap = np.sqrt(np.sum((a - p + np.float32(EPS)) ** 2, axis=-1))
    d_an = np.sqrt(np.sum((a - ng + np.float32(EPS)) ** 2, axis=-1))
    tri = np.maximum(d_ap - d_an + np.float32(MARGIN), np.float32(0.0))
    w = valid[:, None].astype(np.float32)
    denom = max(3.0 * float(valid.sum()), 1.0)
    loss = np.float32(np.sum(tri * w) / denom)
    return np.array(loss, dtype=np.float32)


# revision 14
# speedup vs baseline: 1.2878x; 1.0503x over previous
"""AdaptiveTripletLoss on 8 TRN2 NeuronCores.

Device: the compute-dominant Gram matrix G = E @ E^T (4096x4096x2048)
in fp8 DoubleRow on the PE, f32 PSUM. Exact-cover symmetric
assignment: each core holds 4 blocks [A, C, B, D] plus a 256-col
weight sliver (HW) and computes 4 full block-pairs
(A,A),(A,C),(A,D),(B,C) and one half pair (v,B) — the 4 antipodal
pairs are split by output-row halves between two cores, with the
m-range baked into the host-packed HW region so the program is SPMD.
144 matmuls/core (vs 160 with padded assignments), 4.5 MB streamed
input. The 18 (slot, m) accumulation units run in three PSUM
cohorts (8/8/2); cohort 1 consumes the four streamed chunks as they
arrive. Dummy matmuls pre-warm the PE clock (HAM) during the first
chunk's DMA; redundant Ldweights are deduped post-trace. Outputs
drain per cohort as batched DMAs with 4 KB contiguous dram runs.
Host mirrors the blocks, then does masks/counts, order-statistic
selection, exact d_ap/d_an norms and the masked mean.
"""

import os

import numpy as np
import ml_dtypes

N, D = 4096, 2048
NUM_IDS = 512
N_CORES = 8
MARGIN = 0.3
RATIOS = (0.3, 0.4, 0.3)
EPS = 1e-6

B = 512           # block edge
NB = N // B       # 8x8 block grid
KT = D // 128     # 16 k-tiles
TT = KT // 2      # 8 DoubleRow steps, each contracting 256 k-rows
KC = 4            # input DMA chunks (each = 2 DoubleRow steps)
N_DUMMY = 6       # PE pre-warm matmuls during the first chunk's DMA
NU = 18           # output units: 16 full (slot,m) + 2 half-slot steps

# Exact-cover assignment: per core, blocks [A, C, B, D], half pair
# (v, B) with output rows m in {mbase, mbase+1}. Together the 8 cores
# cover all 36 unordered block pairs exactly once (antipodal pairs
# {0,4},{1,5},{2,6},{3,7} split between two cores by m-half).
ASSIGN = [
    {'A': 4, 'C': 1, 'B': 7, 'D': 3, 'v': 3, 'mbase': 0},
    {'A': 0, 'C': 5, 'B': 4, 'D': 2, 'v': 0, 'mbase': 0},
    {'A': 5, 'C': 2, 'B': 4, 'D': 3, 'v': 0, 'mbase': 2},
    {'A': 2, 'C': 3, 'B': 6, 'D': 1, 'v': 2, 'mbase': 0},
    {'A': 7, 'C': 4, 'B': 6, 'D': 2, 'v': 2, 'mbase': 2},
    {'A': 3, 'C': 0, 'B': 7, 'D': 1, 'v': 3, 'mbase': 2},
    {'A': 6, 'C': 7, 'B': 5, 'D': 0, 'v': 1, 'mbase': 0},
    {'A': 1, 'C': 6, 'B': 5, 'D': 0, 'v': 1, 'mbase': 2},
]

LAST_EXEC_NS = None


def _dedupe_ldweights(nc):
    """Remove Ldweights instructions identical to the immediately
    preceding one (same weights AP + mode): the PE array keeps the
    stationary operand loaded across matmuls, so consecutive matmuls
    sharing lhsT only need the first load."""
    removed = 0
    for fn in nc.m.functions:
        for blk in fn.blocks:
            il = blk.instructions
            prev_key = None
            prev_sync = None
            dels = []
            for idx in range(len(il)):
                ins = il[idx]
                if ins.opcode == "Ldweights":
                    ap = ins.ins[0]
                    key = (
                        getattr(ap, "memref", None),
                        ap.offset,
                        str(ap.ap),
                        str(ins.perf_mode),
                        str(getattr(ins, "is_transpose", None)),
                        str(getattr(ins, "tile_position", None)),
                    )
                    sync = tuple(sorted(ins.sync_dependency_names()))
                    nosync = tuple(sorted(ins.nosync_dependency_names()))
                    if key == prev_key and sync == prev_sync and not nosync:
                        dels.append(idx)
                    else:
                        prev_key = key
                        prev_sync = sync
            for idx in reversed(dels):
                del il[idx]
            removed += len(dels)
    return removed


def _build_gram_kernel():
    import concourse.bacc as bacc
    import concourse.tile as tile
    from concourse import mybir

    nc = bacc.Bacc(None, target_bir_lowering=False)

    f32 = mybir.dt.float32
    bf16 = mybir.dt.bfloat16
    fp8 = mybir.dt.float8e4
    DR = mybir.MatmulPerfMode.DoubleRow

    # Full blocks: per-partition chunk layout [4(q), 2(u), 2(i), 512]
    # fp8 = 8192 B; q order [A, C, B, D].
    blks = nc.declare_dram_parameter("blks", [KC * 128, 4 * 2048], fp8,
                                     isOutput=False)
    # HW weight sliver: [KC, 2(u), 2(i), 256] per partition = 4096 B.
    hwP = nc.declare_dram_parameter("hwP", [128, KC * 1024], fp8,
                                    isOutput=False)
    # Output: row p holds all 18 units' 512 cols (unit-major): a 4-unit
    # drain DMA writes 4 KB contiguous runs per dram row.
    out = nc.declare_dram_parameter("out", [128, NU * B], bf16,
                                    isOutput=True)

    with tile.TileContext(nc) as tc:
        with (
            tc.tile_pool(name="data", bufs=1) as dp,
            tc.tile_pool(name="warm", bufs=1) as wp,
            tc.tile_pool(name="psum", bufs=8, space="PSUM") as pp,
            tc.tile_pool(name="outp", bufs=3) as op,
        ):
            data = dp.tile([128, KC, 4, 2, 2, 512], fp8, name="data")
            hw = dp.tile([128, KC, 2, 2, 256], fp8, name="hw")
            # chunk 0 split so the first matmuls start ~1us earlier
            nc.sync.dma_start(data[:, 0, 0, :, :, :], blks[0:128, 0:2048])
            nc.sync.dma_start(data[:, 0, 1, :, :, :], blks[0:128, 2048:4096])
            nc.sync.dma_start(data[:, 0, 2:4, :, :, :],
                              blks[0:128, 4096:8192])
            for k in range(1, KC):
                nc.sync.dma_start(data[:, k, :, :, :, :],
                                  blks[k * 128:(k + 1) * 128, :])
            nc.sync.dma_start(hw[:, :, :, :, :], hwP[:, :])

            # PE pre-warm: dummy matmuls on a zeroed tile while chunk 0
            # streams in; keeps HAM from throttling the first real mms.
            wl = wp.tile([128, 2, 128], fp8, name="wl")
            wr = wp.tile([128, 2, 512], fp8, name="wr")
            nc.gpsimd.memset(wl[:], 0.0)
            nc.gpsimd.memset(wr[:], 0.0)
            wps = pp.tile([128, B], f32, name="ps")
            for _ in range(N_DUMMY):
                nc.tensor.matmul(wps[:], wl[:], wr[:], start=True, stop=True,
                                 perf_mode=DR)

            # units: 0-3 f0=(A,A) m0-3; 4-7 f1=(A,C); 8-11 f2=(A,D);
            # 12-15 f3=(B,C); 16-17 h_s=(v,B) packed m-halves.
            ps = {}

            def mm(unit, wq, rq, m, t, start, stop, hws=None):
                k, u = t // 2, t % 2
                if hws is None:
                    w_ap = data[:, k, wq, u, :, m * 128:(m + 1) * 128]
                else:
                    w_ap = hw[:, k, u, :, hws * 128:(hws + 1) * 128]
                nc.tensor.matmul(
                    ps[unit][:], w_ap, data[:, k, rq, u, :, :],
                    start=start, stop=stop, perf_mode=DR,
                )

            cast_i = 0

            def drain(units):
                """Cast psums to bf16 and DMA out in one batched
                transfer (4 KB contiguous per dram row)."""
                nonlocal cast_i
                g = op.tile([128, len(units), B], bf16, name="ot")
                for i, u in enumerate(units):
                    eng = (nc.vector, nc.scalar)[cast_i % 2]
                    if eng is nc.scalar:
                        eng.copy(g[:, i, :], ps[u][:])
                    else:
                        eng.tensor_copy(g[:, i, :], ps[u][:])
                    cast_i += 1
                u0 = units[0]
                nc.scalar.dma_start(out[:, u0 * B:(u0 + len(units)) * B],
                                    g[:])

            # Cohort 1: f0 (A,A) units 0-3, f1 (A,C) units 4-7; consume
            # chunks in arrival order.
            for u in range(8):
                ps[u] = pp.tile([128, B], f32, name="ps")
            for c in range(KC):
                for tt in (2 * c, 2 * c + 1):
                    for m in range(4):
                        st, sp = (tt == 0), (tt == TT - 1)
                        mm(0 + m, 0, 0, m, tt, st, sp)
                        mm(4 + m, 0, 1, m, tt, st, sp)
            drain([0, 1, 2, 3])
            drain([4, 5, 6, 7])

            # Cohort 2: f2 (A,D) units 8-11, f3 (B,C) units 12-15.
            for u in range(8, 16):
                ps[u] = pp.tile([128, B], f32, name="ps")
            for t in range(TT):
                for m in range(4):
                    st, sp = (t == 0), (t == TT - 1)
                    mm(8 + m, 0, 3, m, t, st, sp)
                    mm(12 + m, 2, 1, m, t, st, sp)
            drain([8, 9, 10, 11])
            drain([12, 13, 14, 15])

            # Cohort 3: half-slot steps h0, h1 (v,B).
            for u in (16, 17):
                ps[u] = pp.tile([128, B], f32, name="ps")
            for t in range(TT):
                for s in range(2):
                    mm(16 + s, None, 2, None, t, (t == 0), (t == TT - 1),
                       hws=s)
            drain([16, 17])

    _dedupe_ldweights(nc)
    nc.compile()
    return nc


_NC_CACHE = None


def _pack_core(eT8: np.ndarray, w: int):
    """Pack core w's blocks [A, C, B, D] and HW sliver into the device
    layouts. Row k*128+p of blks holds [q, u, i, c] with k-row index
    (4k + 2u + i)*128 + p of eT."""
    g = ASSIGN[w]
    R = eT8.reshape(KC, 2, 2, 128, NB, B)  # [k, u, i, p, block, c]
    qs = [g['A'], g['C'], g['B'], g['D']]
    A = R[:, :, :, :, qs, :]            # [KC, 2, 2, 128, 4, B]
    A = A.transpose(0, 3, 4, 1, 2, 5)   # [KC, 128, 4, 2, 2, B]
    blks = np.ascontiguousarray(A).reshape(KC * 128, 4 * 2048)
    # HW: block v columns mbase*128 .. (mbase+2)*128 -> [p, KC, u, i, 256]
    H = R[:, :, :, :, g['v'], g['mbase'] * 128:(g['mbase'] + 2) * 128]
    H = H.transpose(3, 0, 1, 2, 4)      # [128, KC, 2, 2, 256]
    hwp = np.ascontiguousarray(H).reshape(128, KC * 1024)
    return blks, hwp


def _run_gram(emb: np.ndarray) -> np.ndarray:
    """Run the 8-core symmetric Gram kernel; returns G = emb @ emb.T f32."""
    global _NC_CACHE, LAST_EXEC_NS
    from concourse.bass_utils import run_bass_kernel_spmd

    if _NC_CACHE is None:
        _NC_CACHE = _build_gram_kernel()
    nc = _NC_CACHE

    eT8 = np.ascontiguousarray(emb.T).astype(ml_dtypes.float8_e4m3)
    in_maps = []
    for w in range(N_CORES):
        blks, hwp = _pack_core(eT8, w)
        in_maps.append({"blks": blks, "hwP": hwp})

    trace = bool(int(os.environ.get("KERNEL_TRACE", "0")))
    res = run_bass_kernel_spmd(
        nc, in_maps, core_ids=list(range(N_CORES)), trace=trace
    )
    if res.exec_time_ns is not None:
        LAST_EXEC_NS = res.exec_time_ns

    G = np.empty((N, N), dtype=np.float32)
    for w in range(N_CORES):
        g = ASSIGN[w]
        o = np.asarray(res.results[w]["out"], dtype=np.float32)
        o = o.reshape(128, NU, B)
        slot_pairs = [(g['A'], g['A']), (g['A'], g['C']),
                      (g['A'], g['D']), (g['B'], g['C'])]
        for ui in range(NU):
            if ui < 16:
                wr, rc = slot_pairs[ui // 4]
                m = ui % 4
            else:
                wr, rc = g['v'], g['B']
                m = g['mbase'] + (ui - 16)
            rows = slice(wr * B + m * 128, wr * B + (m + 1) * 128)
            blk = o[:, ui, :]
            G[rows, rc * B:(rc + 1) * B] = blk
            if rc != wr:
                G[rc * B:(rc + 1) * B, rows] = blk.T
    return G


def _sample_js(counts: np.ndarray, us: list) -> np.ndarray:
    """Replicate the reference's f32 sampling math. counts [N] int, us 3x[N]
    f32 uniforms. Returns j ranks [N, 3] int64 (rank into the masked sort)."""
    out = []
    for t, r in enumerate(RATIOS):
        cnt = np.maximum(
            np.int32(1),
            np.floor(counts.astype(np.float32) * np.float32(r)).astype(np.int32),
        )
        j = np.minimum((us[t] * cnt.astype(np.float32)).astype(np.int32), cnt - 1)
        out.append(j.astype(np.int64))
    return np.stack(out, axis=1)


def kernel(embeddings: np.ndarray, labels: np.ndarray) -> np.ndarray:
    emb = np.ascontiguousarray(np.asarray(embeddings, dtype=np.float32))
    lab = np.asarray(labels).astype(np.int64)

    G = _run_gram(emb)

    # Selection keys: within row i, ordering by (sq_j - 2 G[i,j]) equals
    # ordering by distance.
    sq = np.einsum("ij,ij->i", emb, emb).astype(np.float32)

    # Uniforms must match jax.random with key 42 bit-exactly.
    import jax

    with jax.default_device(jax.devices("cpu")[0]):
        skey = jax.random.key(42)
        keys = jax.random.split(skey, 6)
        us = [np.asarray(jax.random.uniform(k, (N,))) for k in keys]

    class_size = np.bincount(lab, minlength=NUM_IDS)
    pos_count = class_size[lab] - 1
    neg_count = N - class_size[lab]
    valid = (pos_count > 0) & (neg_count > 0)

    pos_js = _sample_js(pos_count, us[0:3])  # [N, 3]
    neg_js = _sample_js(neg_count, us[3:6])  # [N, 3]

    # Per-class member lists
    order = np.argsort(lab, kind="stable")
    sorted_lab = lab[order]
    starts = np.searchsorted(sorted_lab, np.arange(NUM_IDS), side="left")
    ends = np.searchsorted(sorted_lab, np.arange(NUM_IDS), side="right")

    pos_idx = np.zeros((N, 3), dtype=np.int64)
    neg_idx = np.zeros((N, 3), dtype=np.int64)
    INF = np.float32(np.inf)

    for i in range(N):
        li = lab[i]
        members = order[starts[li]:ends[li]]
        key_row = sq - 2.0 * G[i]  # f32 [N]
        if valid[i]:
            pos_members = members[members != i]
            pk = key_row[pos_members]
            po = np.argsort(pk, kind="stable")
            pos_idx[i] = pos_members[po[pos_js[i]]]
        # negatives: mask out own class and self
        nk = key_row.copy()
        nk[members] = INF
        nk[i] = INF
        kth = np.unique(neg_js[i])
        part = np.argpartition(nk, kth)
        neg_idx[i] = part[neg_js[i]]

    a = emb[:, None, :]
    p = emb[pos_idx]
    ng = emb[neg_idx]
    d_ap = np.sqrt(np.sum((a - p + np.float32(EPS)) ** 2, axis=-1))
    d_an = np.sqrt(np.sum((a - ng + np.float32(EPS)) ** 2, axis=-1))
    tri = np.maximum(d_ap - d_an + np.float32(MARGIN), np.float32(0.0))
    w = valid[:, None].astype(np.float32)
    denom = max(3.0 * float(valid.sum()), 1.0)
    loss = np.float32(np.sum(tri * w) / denom)
    return np.array(loss, dtype=np.float32)


# revision 19
# speedup vs baseline: 1.3197x; 1.0248x over previous
"""AdaptiveTripletLoss on 8 TRN2 NeuronCores.

Device: the compute-dominant Gram matrix G = E @ E^T (4096x4096x2048)
in fp8 DoubleRow on the PE, f32 PSUM. Exact-cover symmetric
assignment: each core holds 4 blocks [A, C, B, D] plus a 256-col
weight sliver (HW) and computes 4 full block-pairs
(A,A),(A,C),(A,D),(B,C) and one half pair (v,B) — the 4 antipodal
pairs are split by output-row halves between two cores, with the
m-range baked into the host-packed HW region so the program is SPMD.
144 matmuls/core (vs 160 with padded assignments), 4.5 MB streamed
input. The 18 (slot, m) accumulation units run in three PSUM
cohorts (8/8/2); cohort 1 consumes the four streamed chunks as they
arrive. Dummy matmuls pre-warm the PE clock (HAM) during the first
chunk's DMA; redundant Ldweights are deduped post-trace. Outputs
drain per cohort as batched DMAs with 4 KB contiguous dram runs.
Host mirrors the blocks, then does masks/counts, order-statistic
selection, exact d_ap/d_an norms and the masked mean.
"""

import os

import numpy as np
import ml_dtypes

N, D = 4096, 2048
NUM_IDS = 512
N_CORES = 8
MARGIN = 0.3
RATIOS = (0.3, 0.4, 0.3)
EPS = 1e-6

B = 512           # block edge
NB = N // B       # 8x8 block grid
KT = D // 128     # 16 k-tiles
TT = KT // 2      # 8 DoubleRow steps, each contracting 256 k-rows
KC = 4            # input DMA chunks (each = 2 DoubleRow steps)
N_DUMMY = 6       # PE pre-warm matmuls during the first chunk's DMA
NU = 18           # output units: 16 full (slot,m) + 2 half-slot steps

# Exact-cover assignment: per core, blocks [A, C, B, D], half pair
# (v, B) with output rows m in {mbase, mbase+1}. Together the 8 cores
# cover all 36 unordered block pairs exactly once (antipodal pairs
# {0,4},{1,5},{2,6},{3,7} split between two cores by m-half).
ASSIGN = [
    {'A': 4, 'C': 1, 'B': 7, 'D': 3, 'v': 3, 'mbase': 0},
    {'A': 0, 'C': 5, 'B': 4, 'D': 2, 'v': 0, 'mbase': 0},
    {'A': 5, 'C': 2, 'B': 4, 'D': 3, 'v': 0, 'mbase': 2},
    {'A': 2, 'C': 3, 'B': 6, 'D': 1, 'v': 2, 'mbase': 0},
    {'A': 7, 'C': 4, 'B': 6, 'D': 2, 'v': 2, 'mbase': 2},
    {'A': 3, 'C': 0, 'B': 7, 'D': 1, 'v': 3, 'mbase': 2},
    {'A': 6, 'C': 7, 'B': 5, 'D': 0, 'v': 1, 'mbase': 0},
    {'A': 1, 'C': 6, 'B': 5, 'D': 0, 'v': 1, 'mbase': 2},
]

LAST_EXEC_NS = None


def _dedupe_ldweights(nc):
    """Remove Ldweights instructions identical to the immediately
    preceding one (same weights AP + mode): the PE array keeps the
    stationary operand loaded across matmuls, so consecutive matmuls
    sharing lhsT only need the first load."""
    removed = 0
    for fn in nc.m.functions:
        for blk in fn.blocks:
            il = blk.instructions
            prev_key = None
            prev_sync = None
            dels = []
            for idx in range(len(il)):
                ins = il[idx]
                if ins.opcode == "Ldweights":
                    ap = ins.ins[0]
                    key = (
                        getattr(ap, "memref", None),
                        ap.offset,
                        str(ap.ap),
                        str(ins.perf_mode),
                        str(getattr(ins, "is_transpose", None)),
                        str(getattr(ins, "tile_position", None)),
                    )
                    sync = tuple(sorted(ins.sync_dependency_names()))
                    nosync = tuple(sorted(ins.nosync_dependency_names()))
                    if key == prev_key and sync == prev_sync and not nosync:
                        dels.append(idx)
                    else:
                        prev_key = key
                        prev_sync = sync
            for idx in reversed(dels):
                del il[idx]
            removed += len(dels)
    return removed


def _build_gram_kernel():
    import concourse.bacc as bacc
    import concourse.tile as tile
    from concourse import mybir

    nc = bacc.Bacc(None, target_bir_lowering=False)

    f32 = mybir.dt.float32
    bf16 = mybir.dt.bfloat16
    fp8 = mybir.dt.float8e4
    DR = mybir.MatmulPerfMode.DoubleRow

    # Full blocks split by cohort need: cohort 1 reads only A, C; the
    # B, D halves stream after all A/C chunks so cohort 1 is PE-bound
    # from the first chunk on.
    blksAC = nc.declare_dram_parameter("blksAC", [KC * 128, 2 * 2048], fp8,
                                       isOutput=False)
    blksBD = nc.declare_dram_parameter("blksBD", [KC * 128, 2 * 2048], fp8,
                                       isOutput=False)
    # HW weight sliver: [KC, 2(u), 2(i), 256] per partition = 4096 B.
    hwP = nc.declare_dram_parameter("hwP", [128, KC * 1024], fp8,
                                    isOutput=False)
    # Output: row p holds all 18 units' 512 cols (unit-major): a 4-unit
    # drain DMA writes 4 KB contiguous runs per dram row.
    out = nc.declare_dram_parameter("out", [128, NU * B], bf16,
                                    isOutput=True)

    with tile.TileContext(nc) as tc:
        with (
            tc.tile_pool(name="data", bufs=1) as dp,
            tc.tile_pool(name="warm", bufs=1) as wp,
            tc.tile_pool(name="psum", bufs=8, space="PSUM") as pp,
            tc.tile_pool(name="outp", bufs=3) as op,
        ):
            data = dp.tile([128, KC, 4, 2, 2, 512], fp8, name="data")
            hw = dp.tile([128, KC, 2, 2, 256], fp8, name="hw")
            # A/C stream first (cohort 1), chunk 0 split A then C for the
            # earliest possible start; then B/D (cohort 2), then HW.
            nc.sync.dma_start(data[:, 0, 0, :, :, :], blksAC[0:128, 0:2048])
            nc.sync.dma_start(data[:, 0, 1, :, :, :],
                              blksAC[0:128, 2048:4096])
            for k in range(1, KC):
                nc.sync.dma_start(data[:, k, 0:2, :, :, :],
                                  blksAC[k * 128:(k + 1) * 128, :])
            for k in range(KC):
                nc.sync.dma_start(data[:, k, 2:4, :, :, :],
                                  blksBD[k * 128:(k + 1) * 128, :])
            nc.sync.dma_start(hw[:, :, :, :, :], hwP[:, :])

            # PE pre-warm: dummy matmuls on a zeroed tile while chunk 0
            # streams in; keeps HAM from throttling the first real mms.
            wl = wp.tile([128, 2, 128], fp8, name="wl")
            wr = wp.tile([128, 2, 512], fp8, name="wr")
            nc.vector.memset(wl[:], 0.0)
            nc.vector.memset(wr[:], 0.0)
            wps = pp.tile([128, B], f32, name="ps")
            for _ in range(N_DUMMY):
                nc.tensor.matmul(wps[:], wl[:], wr[:], start=True, stop=True,
                                 perf_mode=DR)

            # units: 0-3 f0=(A,A) m0-3; 4-7 f1=(A,C); 8-11 f2=(A,D);
            # 12-15 f3=(B,C); 16-17 h_s=(v,B) packed m-halves.
            ps = {}

            def mm(unit, wq, rq, m, t, start, stop, hws=None):
                k, u = t // 2, t % 2
                if hws is None:
                    w_ap = data[:, k, wq, u, :, m * 128:(m + 1) * 128]
                else:
                    w_ap = hw[:, k, u, :, hws * 128:(hws + 1) * 128]
                nc.tensor.matmul(
                    ps[unit][:], w_ap, data[:, k, rq, u, :, :],
                    start=start, stop=stop, perf_mode=DR,
                )

            cast_i = 0

            def drain(units):
                """Cast psums to bf16 and DMA out in one batched
                transfer (4 KB contiguous per dram row)."""
                nonlocal cast_i
                g = op.tile([128, len(units), B], bf16, name="ot")
                for i, u in enumerate(units):
                    eng = (nc.vector, nc.scalar)[cast_i % 2]
                    if eng is nc.scalar:
                        eng.copy(g[:, i, :], ps[u][:])
                    else:
                        eng.tensor_copy(g[:, i, :], ps[u][:])
                    cast_i += 1
                u0 = units[0]
                nc.scalar.dma_start(out[:, u0 * B:(u0 + len(units)) * B],
                                    g[:])

            # Cohort 1: f0 (A,A) units 0-3, f1 (A,C) units 4-7; consume
            # chunks in arrival order. Phase 0 runs all f0 matmuls
            # before f1 so the A-only prefix starts as soon as the A
            # sub-chunk lands (C arrives ~0.7us later).
            for u in range(8):
                ps[u] = pp.tile([128, B], f32, name="ps")
            for c in range(KC):
                for tt in (2 * c, 2 * c + 1):
                    for m in range(4):
                        st, sp = (tt == 0), (tt == TT - 1)
                        mm(0 + m, 0, 0, m, tt, st, sp)
                        if c > 0:
                            mm(4 + m, 0, 1, m, tt, st, sp)
                if c == 0:
                    for tt in (0, 1):
                        for m in range(4):
                            mm(4 + m, 0, 1, m, tt, (tt == 0), False)
            drain([0, 1, 2, 3])
            drain([4, 5, 6, 7])

            # Cohort 2: f2 (A,D) units 8-11, f3 (B,C) units 12-15.
            for u in range(8, 16):
                ps[u] = pp.tile([128, B], f32, name="ps")
            for t in range(TT):
                for m in range(4):
                    st, sp = (t == 0), (t == TT - 1)
                    mm(8 + m, 0, 3, m, t, st, sp)
                    mm(12 + m, 2, 1, m, t, st, sp)
            drain([8, 9, 10, 11])
            drain([12, 13, 14, 15])

            # Cohort 3: half-slot steps h0, h1 (v,B).
            for u in (16, 17):
                ps[u] = pp.tile([128, B], f32, name="ps")
            for t in range(TT):
                for s in range(2):
                    mm(16 + s, None, 2, None, t, (t == 0), (t == TT - 1),
                       hws=s)
            drain([16, 17])

    _dedupe_ldweights(nc)
    nc.compile()
    return nc


_NC_CACHE = None


def _pack_core(eT8: np.ndarray, w: int):
    """Pack core w's blocks [A, C, B, D] and HW sliver into the device
    layouts. Row k*128+p of blks holds [q, u, i, c] with k-row index
    (4k + 2u + i)*128 + p of eT."""
    g = ASSIGN[w]
    R = eT8.reshape(KC, 2, 2, 128, NB, B)  # [k, u, i, p, block, c]

    def pack2(q0, q1):
        A = R[:, :, :, :, [q0, q1], :]      # [KC, 2, 2, 128, 2, B]
        A = A.transpose(0, 3, 4, 1, 2, 5)   # [KC, 128, 2, 2, 2, B]
        return np.ascontiguousarray(A).reshape(KC * 128, 2 * 2048)

    blks_ac = pack2(g['A'], g['C'])
    blks_bd = pack2(g['B'], g['D'])
    # HW: block v columns mbase*128 .. (mbase+2)*128 -> [p, KC, u, i, 256]
    H = R[:, :, :, :, g['v'], g['mbase'] * 128:(g['mbase'] + 2) * 128]
    H = H.transpose(3, 0, 1, 2, 4)      # [128, KC, 2, 2, 256]
    hwp = np.ascontiguousarray(H).reshape(128, KC * 1024)
    return blks_ac, blks_bd, hwp


def _run_gram(emb: np.ndarray) -> np.ndarray:
    """Run the 8-core symmetric Gram kernel; returns G = emb @ emb.T f32."""
    global _NC_CACHE, LAST_EXEC_NS
    from concourse.bass_utils import run_bass_kernel_spmd

    if _NC_CACHE is None:
        _NC_CACHE = _build_gram_kernel()
    nc = _NC_CACHE

    eT8 = np.ascontiguousarray(emb.T).astype(ml_dtypes.float8_e4m3)
    in_maps = []
    for w in range(N_CORES):
        blks_ac, blks_bd, hwp = _pack_core(eT8, w)
        in_maps.append({"blksAC": blks_ac, "blksBD": blks_bd, "hwP": hwp})

    trace = bool(int(os.environ.get("KERNEL_TRACE", "0")))
    res = run_bass_kernel_spmd(
        nc, in_maps, core_ids=list(range(N_CORES)), trace=trace
    )
    if res.exec_time_ns is not None:
        LAST_EXEC_NS = res.exec_time_ns

    G = np.empty((N, N), dtype=np.float32)
    for w in range(N_CORES):
        g = ASSIGN[w]
        o = np.asarray(res.results[w]["out"], dtype=np.float32)
        o = o.reshape(128, NU, B)
        slot_pairs = [(g['A'], g['A']), (g['A'], g['C']),
                      (g['A'], g['D']), (g['B'], g['C'])]
        for ui in range(NU):
            if ui < 16:
                wr, rc = slot_pairs[ui // 4]
                m = ui % 4
            else:
                wr, rc = g['v'], g['B']
                m = g['mbase'] + (ui - 16)
            rows = slice(wr * B + m * 128, wr * B + (m + 1) * 128)
            blk = o[:, ui, :]
            G[rows, rc * B:(rc + 1) * B] = blk
            if rc != wr:
                G[rc * B:(rc + 1) * B, rows] = blk.T
    return G


def _sample_js(counts: np.ndarray, us: list) -> np.ndarray:
    """Replicate the reference's f32 sampling math. counts [N] int, us 3x[N]
    f32 uniforms. Returns j ranks [N, 3] int64 (rank into the masked sort)."""
    out = []
    for t, r in enumerate(RATIOS):
        cnt = np.maximum(
            np.int32(1),
            np.floor(counts.astype(np.float32) * np.float32(r)).astype(np.int32),
        )
        j = np.minimum((us[t] * cnt.astype(np.float32)).astype(np.int32), cnt - 1)
        out.append(j.astype(np.int64))
    return np.stack(out, axis=1)


def kernel(embeddings: np.ndarray, labels: np.ndarray) -> np.ndarray:
    emb = np.ascontiguousarray(np.asarray(embeddings, dtype=np.float32))
    lab = np.asarray(labels).astype(np.int64)

    G = _run_gram(emb)

    # Selection keys: within row i, ordering by (sq_j - 2 G[i,j]) equals
    # ordering by distance.
    sq = np.einsum("ij,ij->i", emb, emb).astype(np.float32)

    # Uniforms must match jax.random with key 42 bit-exactly.
    import jax

    with jax.default_device(jax.devices("cpu")[0]):
        skey = jax.random.key(42)
        keys = jax.random.split(skey, 6)
        us = [np.asarray(jax.random.uniform(k, (N,))) for k in keys]

    class_size = np.bincount(lab, minlength=NUM_IDS)
    pos_count = class_size[lab] - 1
    neg_count = N - class_size[lab]
    valid = (pos_count > 0) & (neg_count > 0)

    pos_js = _sample_js(pos_count, us[0:3])  # [N, 3]
    neg_js = _sample_js(neg_count, us[3:6])  # [N, 3]

    # Per-class member lists
    order = np.argsort(lab, kind="stable")
    sorted_lab = lab[order]
    starts = np.searchsorted(sorted_lab, np.arange(NUM_IDS), side="left")
    ends = np.searchsorted(sorted_lab, np.arange(NUM_IDS), side="right")

    pos_idx = np.zeros((N, 3), dtype=np.int64)
    neg_idx = np.zeros((N, 3), dtype=np.int64)
    INF = np.float32(np.inf)

    for i in range(N):
        li = lab[i]
        members = order[starts[li]:ends[li]]
        key_row = sq - 2.0 * G[i]  # f32 [N]
        if valid[i]:
            pos_members = members[members != i]
            pk = key_row[pos_members]
            po = np.argsort(pk, kind="stable")
            pos_idx[i] = pos_members[po[pos_js[i]]]
        # negatives: mask out own class and self
        nk = key_row.copy()
        nk[members] = INF
        nk[i] = INF
        kth = np.unique(neg_js[i])
        part = np.argpartition(nk, kth)
        neg_idx[i] = part[neg_js[i]]

    a = emb[:, None, :]
    p = emb[pos_idx]
    ng = emb[neg_idx]
    d_ap = np.sqrt(np.sum((a - p + np.float32(EPS)) ** 2, axis=-1))
    d_an = np.sqrt(np.sum((a - ng + np.float32(EPS)) ** 2, axis=-1))
    tri = np.maximum(d_ap - d_an + np.float32(MARGIN), np.float32(0.0))
    w = valid[:, None].astype(np.float32)
    denom = max(3.0 * float(valid.sum()), 1.0)
    loss = np.float32(np.sum(tri * w) / denom)
    return np.array(loss, dtype=np.float32)


# revision 25
# speedup vs baseline: 1.4150x; 1.0722x over previous
"""AdaptiveTripletLoss on 8 TRN2 NeuronCores.

Device: the compute-dominant Gram matrix G = E @ E^T (4096x4096x2048)
in fp8 DoubleRow on the PE, f32 PSUM. Exact-cover symmetric
assignment: each core holds 4 blocks [A, C, B, D] plus a 256-col
weight sliver (HW) and computes 4 full block-pairs
(A,A),(A,C),(A,D),(B,C) and one half pair (v,B) — the 4 antipodal
pairs are split by output-row halves between two cores, with the
m-range baked into the host-packed HW region so the program is SPMD.
144 matmuls/core (vs 160 with padded assignments), 4.5 MB streamed
input. The 18 (slot, m) accumulation units run in three PSUM
cohorts (8/8/2); cohort 1 consumes the four streamed chunks as they
arrive. Dummy matmuls pre-warm the PE clock (HAM) during the first
chunk's DMA; redundant Ldweights are deduped post-trace. Outputs
drain per cohort as batched DMAs with 4 KB contiguous dram runs.
Host mirrors the blocks, then does masks/counts, order-statistic
selection, exact d_ap/d_an norms and the masked mean.
"""

import os

import numpy as np
import ml_dtypes

N, D = 4096, 2048
NUM_IDS = 512
N_CORES = 8
MARGIN = 0.3
RATIOS = (0.3, 0.4, 0.3)
EPS = 1e-6

B = 512           # block edge
NB = N // B       # 8x8 block grid
KT = D // 128     # 16 k-tiles
TT = KT // 2      # 8 DoubleRow steps, each contracting 256 k-rows
KC = 4            # input DMA chunks (each = 2 DoubleRow steps)
N_DUMMY = 3       # PE pre-warm matmuls during the first chunk's DMA
NU = 18           # output units: 16 full (slot,m) + 2 half-slot steps

# Exact-cover assignment: per core, blocks [A, C, B, D], half pair
# (v, B) with output rows m in {mbase, mbase+1}. Together the 8 cores
# cover all 36 unordered block pairs exactly once (antipodal pairs
# {0,4},{1,5},{2,6},{3,7} split between two cores by m-half).
ASSIGN = [
    {'A': 4, 'C': 1, 'B': 7, 'D': 3, 'v': 3, 'mbase': 0},
    {'A': 0, 'C': 5, 'B': 4, 'D': 2, 'v': 0, 'mbase': 0},
    {'A': 5, 'C': 2, 'B': 4, 'D': 3, 'v': 0, 'mbase': 2},
    {'A': 2, 'C': 3, 'B': 6, 'D': 1, 'v': 2, 'mbase': 0},
    {'A': 7, 'C': 4, 'B': 6, 'D': 2, 'v': 2, 'mbase': 2},
    {'A': 3, 'C': 0, 'B': 7, 'D': 1, 'v': 3, 'mbase': 2},
    {'A': 6, 'C': 7, 'B': 5, 'D': 0, 'v': 1, 'mbase': 0},
    {'A': 1, 'C': 6, 'B': 5, 'D': 0, 'v': 1, 'mbase': 2},
]

LAST_EXEC_NS = None


def _dedupe_ldweights(nc):
    """Remove Ldweights instructions identical to the immediately
    preceding one (same weights AP + mode): the PE array keeps the
    stationary operand loaded across matmuls, so consecutive matmuls
    sharing lhsT only need the first load."""
    removed = 0
    for fn in nc.m.functions:
        for blk in fn.blocks:
            il = blk.instructions
            prev_key = None
            prev_sync = None
            dels = []
            for idx in range(len(il)):
                ins = il[idx]
                if ins.opcode == "Ldweights":
                    ap = ins.ins[0]
                    key = (
                        getattr(ap, "memref", None),
                        ap.offset,
                        str(ap.ap),
                        str(ins.perf_mode),
                        str(getattr(ins, "is_transpose", None)),
                        str(getattr(ins, "tile_position", None)),
                    )
                    sync = tuple(sorted(ins.sync_dependency_names()))
                    nosync = tuple(sorted(ins.nosync_dependency_names()))
                    if key == prev_key and sync == prev_sync and not nosync:
                        dels.append(idx)
                    else:
                        prev_key = key
                        prev_sync = sync
            for idx in reversed(dels):
                del il[idx]
            removed += len(dels)
    return removed


def _build_gram_kernel():
    import concourse.bacc as bacc
    import concourse.tile as tile
    from concourse import mybir

    nc = bacc.Bacc(None, target_bir_lowering=False)

    f32 = mybir.dt.float32
    bf16 = mybir.dt.bfloat16
    fp8 = mybir.dt.float8e4
    DR = mybir.MatmulPerfMode.DoubleRow

    # Full blocks split by cohort need: cohort 1 reads only A, C; the
    # B, D halves stream after all A/C chunks so cohort 1 is PE-bound
    # from the first chunk on.
    blksAC = nc.declare_dram_parameter("blksAC", [KC * 128, 2 * 2048], fp8,
                                       isOutput=False)
    blksBD = nc.declare_dram_parameter("blksBD", [KC * 128, 2 * 2048], fp8,
                                       isOutput=False)
    # HW weight sliver: [KC, 2(u), 2(i), 256] per partition = 4096 B.
    hwP = nc.declare_dram_parameter("hwP", [128, KC * 1024], fp8,
                                    isOutput=False)
    # Output: row p holds all 18 units' 512 cols (unit-major): a 4-unit
    # drain DMA writes 4 KB contiguous runs per dram row.
    out = nc.declare_dram_parameter("out", [128, NU * B], bf16,
                                    isOutput=True)

    with tile.TileContext(nc) as tc:
        with (
            tc.tile_pool(name="data", bufs=1) as dp,
            tc.tile_pool(name="warm", bufs=1) as wp,
            tc.tile_pool(name="psum", bufs=8, space="PSUM") as pp,
            tc.tile_pool(name="outp", bufs=3) as op,
        ):
            data = dp.tile([128, KC, 4, 2, 2, 512], fp8, name="data")
            hw = dp.tile([128, KC, 2, 2, 256], fp8, name="hw")
            # A/C stream first (cohort 1), chunk 0 split A then C for the
            # earliest possible start; then B/D (cohort 2), then HW.
            nc.sync.dma_start(data[:, 0, 0, :, :, :], blksAC[0:128, 0:2048])
            nc.sync.dma_start(data[:, 0, 1, :, :, :],
                              blksAC[0:128, 2048:4096])
            for k in range(1, KC):
                nc.sync.dma_start(data[:, k, 0:2, :, :, :],
                                  blksAC[k * 128:(k + 1) * 128, :])
            for k in range(KC):
                nc.sync.dma_start(data[:, k, 2:4, :, :, :],
                                  blksBD[k * 128:(k + 1) * 128, :])
            nc.sync.dma_start(hw[:, :, :, :, :], hwP[:, :])

            # PE pre-warm: dummy matmuls on a zeroed tile while chunk 0
            # streams in; keeps HAM from throttling the first real mms.
            wl = wp.tile([128, 2, 128], fp8, name="wl")
            wr = wp.tile([128, 2, 512], fp8, name="wr")
            nc.vector.memset(wl[:], 0.0)
            nc.vector.memset(wr[:], 0.0)
            wps = pp.tile([128, B], f32, name="ps")
            for _ in range(N_DUMMY):
                nc.tensor.matmul(wps[:], wl[:], wr[:], start=True, stop=True,
                                 perf_mode=DR)

            # units: 0-3 f0=(A,A) m0-3; 4-7 f1=(A,C); 8-11 f2=(A,D);
            # 12-15 f3=(B,C); 16-17 h_s=(v,B) packed m-halves.
            ps = {}

            def mm(unit, wq, rq, m, t, start, stop, hws=None, c0=0):
                # c0: first rhs column (diagonal blocks are symmetric, so
                # f0's matmul m only needs cols >= m*128; host mirrors).
                k, u = t // 2, t % 2
                if hws is None:
                    w_ap = data[:, k, wq, u, :, m * 128:(m + 1) * 128]
                else:
                    w_ap = hw[:, k, u, :, hws * 128:(hws + 1) * 128]
                nc.tensor.matmul(
                    ps[unit][:, c0:], w_ap, data[:, k, rq, u, :, c0:],
                    start=start, stop=stop, perf_mode=DR,
                )

            cast_i = 0

            def drain(units, ring=None):
                """Cast psums to bf16 and DMA out in one batched
                transfer (4 KB contiguous per dram row)."""
                nonlocal cast_i
                g = op.tile([128, len(units), B], bf16, name="ot")
                for i, u in enumerate(units):
                    eng = (nc.vector, nc.scalar)[cast_i % 2]
                    if eng is nc.scalar:
                        eng.copy(g[:, i, :], ps[u][:])
                    else:
                        eng.tensor_copy(g[:, i, :], ps[u][:])
                    cast_i += 1
                u0 = units[0]
                (ring or nc.scalar).dma_start(
                    out[:, u0 * B:(u0 + len(units)) * B], g[:])

            # Cohort 1: f0 (A,A) units 0-3, f1 (A,C) units 4-7; consume
            # chunks in arrival order. Phase 0 runs all f0 matmuls
            # before f1 so the A-only prefix starts as soon as the A
            # sub-chunk lands (C arrives ~0.7us later).
            for u in range(8):
                ps[u] = pp.tile([128, B], f32, name="ps")
            for c in range(KC):
                for tt in (2 * c, 2 * c + 1):
                    for m in range(4):
                        st, sp = (tt == 0), (tt == TT - 1)
                        mm(0 + m, 0, 0, m, tt, st, sp, c0=m * 128)
                        if c > 0:
                            mm(4 + m, 0, 1, m, tt, st, sp)
                if c == 0:
                    for tt in (0, 1):
                        for m in range(4):
                            mm(4 + m, 0, 1, m, tt, (tt == 0), False)
            drain([0, 1, 2, 3])
            drain([4, 5, 6, 7])

            # Cohort 2: f2 (A,D) units 8-11, f3 (B,C) units 12-15.
            for u in range(8, 16):
                ps[u] = pp.tile([128, B], f32, name="ps")
            for t in range(TT):
                for m in range(4):
                    st, sp = (t == 0), (t == TT - 1)
                    mm(8 + m, 0, 3, m, t, st, sp)
                    mm(12 + m, 2, 1, m, t, st, sp)
            drain([8, 9, 10, 11], ring=nc.sync)
            drain([12, 13, 14, 15])

            # Cohort 3: half-slot steps h0, h1 (v,B); drains split
            # across both DMA rings to shorten the tail.
            for u in (16, 17):
                ps[u] = pp.tile([128, B], f32, name="ps")
            for t in range(TT):
                for s in range(2):
                    mm(16 + s, None, 2, None, t, (t == 0), (t == TT - 1),
                       hws=s)
            drain([16], ring=nc.sync)
            drain([17])

    _dedupe_ldweights(nc)
    nc.compile()
    return nc


_NC_CACHE = None


def _pack_core(eT8: np.ndarray, w: int):
    """Pack core w's blocks [A, C, B, D] and HW sliver into the device
    layouts. Row k*128+p of blks holds [q, u, i, c] with k-row index
    (4k + 2u + i)*128 + p of eT."""
    g = ASSIGN[w]
    R = eT8.reshape(KC, 2, 2, 128, NB, B)  # [k, u, i, p, block, c]

    def pack2(q0, q1):
        A = R[:, :, :, :, [q0, q1], :]      # [KC, 2, 2, 128, 2, B]
        A = A.transpose(0, 3, 4, 1, 2, 5)   # [KC, 128, 2, 2, 2, B]
        return np.ascontiguousarray(A).reshape(KC * 128, 2 * 2048)

    blks_ac = pack2(g['A'], g['C'])
    blks_bd = pack2(g['B'], g['D'])
    # HW: block v columns mbase*128 .. (mbase+2)*128 -> [p, KC, u, i, 256]
    H = R[:, :, :, :, g['v'], g['mbase'] * 128:(g['mbase'] + 2) * 128]
    H = H.transpose(3, 0, 1, 2, 4)      # [128, KC, 2, 2, 256]
    hwp = np.ascontiguousarray(H).reshape(128, KC * 1024)
    return blks_ac, blks_bd, hwp


def _run_gram(emb: np.ndarray) -> np.ndarray:
    """Run the 8-core symmetric Gram kernel; returns G = emb @ emb.T f32."""
    global _NC_CACHE, LAST_EXEC_NS
    from concourse.bass_utils import run_bass_kernel_spmd

    if _NC_CACHE is None:
        _NC_CACHE = _build_gram_kernel()
    nc = _NC_CACHE

    eT8 = np.ascontiguousarray(emb.T).astype(ml_dtypes.float8_e4m3)
    in_maps = []
    for w in range(N_CORES):
        blks_ac, blks_bd, hwp = _pack_core(eT8, w)
        in_maps.append({"blksAC": blks_ac, "blksBD": blks_bd, "hwP": hwp})

    trace = bool(int(os.environ.get("KERNEL_TRACE", "0")))
    res = run_bass_kernel_spmd(
        nc, in_maps, core_ids=list(range(N_CORES)), trace=trace
    )
    if res.exec_time_ns is not None:
        LAST_EXEC_NS = res.exec_time_ns

    G = np.empty((N, N), dtype=np.float32)
    for w in range(N_CORES):
        g = ASSIGN[w]
        o = np.asarray(res.results[w]["out"], dtype=np.float32)
        o = o.reshape(128, NU, B)
        slot_pairs = [(g['A'], g['A']), (g['A'], g['C']),
                      (g['A'], g['D']), (g['B'], g['C'])]
        for ui in range(NU):
            if ui < 16:
                wr, rc = slot_pairs[ui // 4]
                m = ui % 4
            else:
                wr, rc = g['v'], g['B']
                m = g['mbase'] + (ui - 16)
            rows = slice(wr * B + m * 128, wr * B + (m + 1) * 128)
            blk = o[:, ui, :]
            if ui < 4:
                # diagonal block: device computed only cols >= m*128;
                # mirror the rectangle into both triangles.
                sub = blk[:, m * 128:]
                G[rows, rc * B + m * 128:(rc + 1) * B] = sub
                G[rc * B + m * 128:(rc + 1) * B, rows] = sub.T
            else:
                G[rows, rc * B:(rc + 1) * B] = blk
                if rc != wr:
                    G[rc * B:(rc + 1) * B, rows] = blk.T
    return G


def _sample_js(counts: np.ndarray, us: list) -> np.ndarray:
    """Replicate the reference's f32 sampling math. counts [N] int, us 3x[N]
    f32 uniforms. Returns j ranks [N, 3] int64 (rank into the masked sort)."""
    out = []
    for t, r in enumerate(RATIOS):
        cnt = np.maximum(
            np.int32(1),
            np.floor(counts.astype(np.float32) * np.float32(r)).astype(np.int32),
        )
        j = np.minimum((us[t] * cnt.astype(np.float32)).astype(np.int32), cnt - 1)
        out.append(j.astype(np.int64))
    return np.stack(out, axis=1)


def kernel(embeddings: np.ndarray, labels: np.ndarray) -> np.ndarray:
    emb = np.ascontiguousarray(np.asarray(embeddings, dtype=np.float32))
    lab = np.asarray(labels).astype(np.int64)

    G = _run_gram(emb)

    # Selection keys: within row i, ordering by (sq_j - 2 G[i,j]) equals
    # ordering by distance.
    sq = np.einsum("ij,ij->i", emb, emb).astype(np.float32)

    # Uniforms must match jax.random with key 42 bit-exactly.
    import jax

    with jax.default_device(jax.devices("cpu")[0]):
        skey = jax.random.key(42)
        keys = jax.random.split(skey, 6)
        us = [np.asarray(jax.random.uniform(k, (N,))) for k in keys]

    class_size = np.bincount(lab, minlength=NUM_IDS)
    pos_count = class_size[lab] - 1
    neg_count = N - class_size[lab]
    valid = (pos_count > 0) & (neg_count > 0)

    pos_js = _sample_js(pos_count, us[0:3])  # [N, 3]
    neg_js = _sample_js(neg_count, us[3:6])  # [N, 3]

    # Per-class member lists
    order = np.argsort(lab, kind="stable")
    sorted_lab = lab[order]
    starts = np.searchsorted(sorted_lab, np.arange(NUM_IDS), side="left")
    ends = np.searchsorted(sorted_lab, np.arange(NUM_IDS), side="right")

    pos_idx = np.zeros((N, 3), dtype=np.int64)
    neg_idx = np.zeros((N, 3), dtype=np.int64)
    INF = np.float32(np.inf)

    for i in range(N):
        li = lab[i]
        members = order[starts[li]:ends[li]]
        key_row = sq - 2.0 * G[i]  # f32 [N]
        if valid[i]:
            pos_members = members[members != i]
            pk = key_row[pos_members]
            po = np.argsort(pk, kind="stable")
            pos_idx[i] = pos_members[po[pos_js[i]]]
        # negatives: mask out own class and self
        nk = key_row.copy()
        nk[members] = INF
        nk[i] = INF
        kth = np.unique(neg_js[i])
        part = np.argpartition(nk, kth)
        neg_idx[i] = part[neg_js[i]]

    a = emb[:, None, :]
    p = emb[pos_idx]
    ng = emb[neg_idx]
    d_ap = np.sqrt(np.sum((a - p + np.float32(EPS)) ** 2, axis=-1))
    d_an = np.sqrt(np.sum((a - ng + np.float32(EPS)) ** 2, axis=-1))
    tri = np.maximum(d_ap - d_an + np.float32(MARGIN), np.float32(0.0))
    w = valid[:, None].astype(np.float32)
    denom = max(3.0 * float(valid.sum()), 1.0)
    loss = np.float32(np.sum(tri * w) / denom)
    return np.array(loss, dtype=np.float32)
